# revision 49
# baseline (speedup 1.0000x reference)
"""Trainium2 Bass kernel for 16-head self-attention (b=2, n=2048, dm=1024, dh=64).

Sharding: each of 8 cores owns (batch g = c//4, sequence block r = c%4).
A core computes K,V for its batch's FULL sequence (replicated across the 4
cores of that batch -- avoids cross-core collectives), attention for all 16
heads restricted to its 512 query rows, and the output projection for those
rows.  Per-core outputs are disjoint [512, 1024] slices of the final
[2, 2048, 1024]; the host concatenates.

Host passes x^T (dm-major) ROTATED by the core's row offset, so every core's
query slice is columns 0:512 of its own x^T -- the SPMD program is identical
across cores.  Attention is permutation-invariant over keys, so the rotation
does not change the result.

v3: single hand-woven schedule.  The scalar engine's exp stream (16 heads x
8 instructions x ~1.1us over [128, 2*512] PSUM pairs) is the attention-phase
floor, and the PE drops to its mid p-state (1.2 GHz, 2x slower matmuls)
whenever it idles waiting on it.  So instead of phased projections followed
by attention, only Q(ib=0)+K(ib=0) run up front; attention pair 0 starts
~15us in, and ALL remaining work -- Q(1..7), K(1..7) per 512-key chunk,
V units (one [128 keys x 512 inner-cols] block each), and the per-pair
rank-128 output-projection updates -- is interleaved between the S/exp/AV
groups as PE filler with just-in-time deadlines (K(p) before pair p,
V(kb,ic) blocks two kb ahead of the consuming AV, Wq freed after Q(7) to
make room for Wo).  This keeps ACT busy from the start, the PE dense (and
hence at full clock), and leaves only the last pair's softmax tail exposed.

Other structure (from v2): all-bf16 operands; K^T and V entirely in SBUF;
V stored [128, kb, head, 65] with a ones column so the AV matmul emits the
softmax denominator for free; exact DVE reciprocal (approx_fast
miscompiles on this toolchain; ACT Ln/Exp reciprocal costs two 1.3us table
reloads); gpsimd partition_broadcast for the 1/denom cross-partition
broadcast (gpsimd cannot touch PSUM, so the normalize multiply and the
accumulator adds stay on DVE).
"""

import sys

for _p in ("/opt/trn_rl_repo", "/root/.axon_site/_ro/trn_rl_repo"):
    if _p not in sys.path:
        sys.path.append(_p)

import numpy as np

B = 2
N = 2048
DM = 1024
H = 16
DH = 64
INNER = H * DH  # 1024
NCORES = 8
QR = 512  # query rows per core
SCALE = DH ** -0.5

_cached = {}


def _build():
    import contextlib
    import concourse.bacc as bacc
    import concourse.tile as tile
    import concourse.mybir as mybir

    f32 = mybir.dt.float32
    bf16 = mybir.dt.bfloat16
    Exp = mybir.ActivationFunctionType.Exp

    nc = bacc.Bacc("TRN2", target_bir_lowering=False, debug=False,
                   enable_asserts=False)

    xT_d = nc.dram_tensor("xT", [DM, N], bf16, kind="ExternalInput").ap()
    xq_d = nc.dram_tensor("xq", [DM, QR], bf16, kind="ExternalInput").ap()
    Wq_d = nc.dram_tensor("Wq", [DM, INNER], bf16, kind="ExternalInput").ap()
    Wk_d = nc.dram_tensor("Wk", [DM, INNER], bf16, kind="ExternalInput").ap()
    Wv_d = nc.dram_tensor("Wv", [DM, INNER], bf16, kind="ExternalInput").ap()
    Wo_d = nc.dram_tensor("Wo", [INNER, DM], bf16, kind="ExternalInput").ap()
    bo_d = nc.dram_tensor("bo", [DM], f32, kind="ExternalInput").ap()
    out_d = nc.dram_tensor("out", [QR, DM], f32, kind="ExternalOutput").ap()

    A = DM // 128      # 8 dm blocks
    IB = INNER // 128  # 8 inner blocks
    KB = N // 128      # 16 key blocks
    KC = N // 512      # 4 key chunks
    QB = QR // 128     # 4 query blocks

    with tile.TileContext(nc) as tc, \
         nc.allow_low_precision(reason="bf16 matmul pipeline, validated e2e"), \
         contextlib.ExitStack() as ctx:
        persist = ctx.enter_context(tc.tile_pool(name="persist", bufs=1))
        xT_sb = persist.tile([128, A, N], bf16)
        xq_sb = persist.tile([128, A, QR], bf16)
        QT_z = persist.tile([128, IB, 2, QR], bf16)
        KT_sb = persist.tile([128, IB, N], bf16)    # K^T [inner, keys]
        V_sb = persist.tile([128, KB, H, 65], bf16)  # V + ones col per head
        OT_sb = persist.tile([128, IB, QR], bf16)   # O^T  [inner, q]
        out_acc = persist.tile([128, QB, DM], f32)  # out rows accumulator
        bo_sb = persist.tile([128, DM], f32)
        onef = persist.tile([128, 1], f32)
        zerof = persist.tile([128, 1], f32)

        nc.vector.memset(onef, 1.0)
        nc.vector.memset(zerof, 0.0)
        nc.vector.tensor_copy(
            out=QT_z[:, :, :, :],
            in_=zerof.unsqueeze(1).unsqueeze(1).to_broadcast(
                [128, IB, 2, QR]))
        nc.vector.tensor_copy(
            out=V_sb[:, :, :, 64:65],
            in_=onef.unsqueeze(1).unsqueeze(1).to_broadcast([128, KB, H, 1]))
        nc.gpsimd.dma_start(
            out=bo_sb, in_=bo_d.unsqueeze(0).to_broadcast([128, DM]))

        dram = ctx.enter_context(
            tc.tile_pool(name="dram", bufs=1, space="DRAM"))
        k_stage = dram.tile([IB, 128, QR], bf16)
        k_gath = dram.tile([IB // 2, 4, 2, 128, QR], bf16)

        xT_r = xT_d.rearrange("(a p) n -> a p n", p=128)
        Wq_r = Wq_d.rearrange("(a p) i -> a p i", p=128)
        Wk_r = Wk_d.rearrange("(a p) i -> a p i", p=128)
        Wv_r = Wv_d.rearrange("(a p) i -> a p i", p=128)
        Wo_r = Wo_d.rearrange("(ib p) d -> ib p d", p=128)

        wpool = ctx.enter_context(tc.tile_pool(name="wqk", bufs=1))
        Wk_sb = wpool.tile([128, A, INNER], bf16)
        Wv_sb = wpool.tile([128, A, INNER], bf16)
        pes = ctx.enter_context(tc.tile_pool(name="pb_es", bufs=1))
        psm = ctx.enter_context(tc.tile_pool(name="pb_sm", bufs=1))
        psb = ctx.enter_context(tc.tile_pool(name="pb_ps", bufs=1,
                                             space="PSUM"))
        # Wq lives in its own pool, allocated last, so releasing it after
        # Q(7) frees the top of the SBUF stack for Wo.
        wq_pool = tc.alloc_tile_pool(name="wq", bufs=1)
        Wq_sb = wq_pool.tile([128, A, INNER], bf16)

        xq_r = xq_d.rearrange("(a p) n -> a p n", p=128)
        for a in range(A):
            nc.sync.dma_start(out=xq_sb[:, a, :], in_=xq_r[a])
        for a in range(A):
            nc.sync.dma_start(out=Wq_sb[:, a, :], in_=Wq_r[a])
        for a in range(A):
            nc.sync.dma_start(out=Wk_sb[:, a, :], in_=Wk_r[a])
        for a in range(A):
            nc.sync.dma_start(out=xT_sb[:, a, :], in_=xT_r[a])
        for a in range(A):
            nc.sync.dma_start(out=Wv_sb[:, a, :], in_=Wv_r[a])

        wo_state = {}

        # ---------------- fill units ----------------
        def q_unit(ib):
            qp = psb.tile([128, QR], f32, tag="proj", bufs=2, name="qp")
            for a in range(A):
                nc.tensor.matmul(
                    out=qp,
                    lhsT=Wq_sb[:, a, ib * 128:(ib + 1) * 128],
                    rhs=xq_sb[:, a, :],
                    start=(a == 0), stop=(a == A - 1))
            nc.vector.tensor_copy(out=QT_z[0:64, ib, 0, :], in_=qp[0:64, :])
            nc.vector.tensor_copy(out=QT_z[64:128, ib, 1, :],
                                  in_=qp[64:128, :])

        def k_unit(ib):
            kp = psb.tile([128, 512], f32, tag="proj", bufs=2, name="kp")
            for a in range(A):
                nc.tensor.matmul(
                    out=kp,
                    lhsT=Wk_sb[:, a, ib * 128:(ib + 1) * 128],
                    rhs=xq_sb[:, a, :],
                    start=(a == 0), stop=(a == A - 1))
            nc.vector.tensor_copy(out=KT_sb[:, ib, 0:QR], in_=kp)
            nc.sync.dma_start(out=k_stage[ib], in_=KT_sb[:, ib, 0:QR])

        def k_gather(j):
            nc.gpsimd.collective_compute(
                kind="AllGather", op=mybir.AluOpType.bypass,
                replica_groups=[[0, 1, 2, 3], [4, 5, 6, 7]],
                ins=[k_stage[2 * j:2 * j + 2]], outs=[k_gath[j]])
            for i in range(2):
                nc.sync.dma_start(
                    out=KT_sb[:, 2 * j + i, :],
                    in_=k_gath[j][:, i, :, :].rearrange("s p n -> p s n"))

        def v_unit(kb, ic):
            vp = psb.tile([128, 512], f32, tag="proj", bufs=2, name="vp")
            for a in range(A):
                nc.tensor.matmul(
                    out=vp,
                    lhsT=xT_sb[:, a, kb * 128:(kb + 1) * 128],
                    rhs=Wv_sb[:, a, ic * 512:(ic + 1) * 512],
                    start=(a == 0), stop=(a == A - 1))
            nc.vector.tensor_copy(
                out=V_sb[:, kb, ic * 8:(ic + 1) * 8, 0:64], in_=vp)

        def swap_wq_for_wo():
            wq_pool.release()
            wo_pool = tc.alloc_tile_pool(name="wo", bufs=1)
            Wo_sb = wo_pool.tile([128, IB, DM], bf16)
            for ib in range(IB):
                nc.sync.dma_start(out=Wo_sb[:, ib, :], in_=Wo_r[ib])
            wo_state["Wo_sb"] = Wo_sb
            wo_state["pool"] = wo_pool

        def o_unit(hp, qb, dc):
            Wo_sb = wo_state["Wo_sb"]
            outp = psb.tile([128, 512], f32, tag="proj", bufs=2, name="outp")
            nc.tensor.matmul(
                out=outp,
                lhsT=OT_sb[:, hp, qb * 128:(qb + 1) * 128],
                rhs=Wo_sb[:, hp, dc * 512:(dc + 1) * 512],
                start=True, stop=True)
            dst = out_acc[:, qb, dc * 512:(dc + 1) * 512]
            if hp == 0:
                nc.vector.tensor_add(
                    dst, outp, bo_sb[:, dc * 512:(dc + 1) * 512])
            else:
                nc.vector.tensor_add(dst, outp, dst)
            if hp == IB - 1:
                nc.sync.dma_start(
                    out=out_d[qb * 128:(qb + 1) * 128,
                              dc * 512:(dc + 1) * 512],
                    in_=dst)

        def o_tail_chunk(hh, qb, final):
            # pair-7 per-head rank-64 update for one query block
            Wo_sb = wo_state["Wo_sb"]
            for dc in range(2):
                outp = psb.tile([128, 512], f32, tag="proj", bufs=2,
                                name="outp")
                nc.tensor.matmul(
                    out=outp,
                    lhsT=OT_sb[hh * 64:(hh + 1) * 64, 7,
                               qb * 128:(qb + 1) * 128],
                    rhs=Wo_sb[hh * 64:(hh + 1) * 64, 7,
                              dc * 512:(dc + 1) * 512],
                    start=True, stop=True)
                dst = out_acc[:, qb, dc * 512:(dc + 1) * 512]
                nc.vector.tensor_add(dst, outp, dst)
                if final:
                    nc.sync.dma_start(
                        out=out_d[qb * 128:(qb + 1) * 128,
                                  dc * 512:(dc + 1) * 512],
                        in_=dst)

        # ---------------- fill schedule ----------------
        # hooks[(h, g)] -> list of zero-arg fill closures emitted after
        # S/exp/AV group g of head h.  Deadlines honored by construction:
        #   K(p), Q(p) complete before pair p;  V(kb, ic) lands >= 2 kb
        #   ahead of the first consuming AV (head 0 for ic0, head 8 for
        #   ic1);  out-proj updates only need OT rows finished a pair ago.
        from collections import defaultdict
        hooks = defaultdict(list)

        def at(h, g, fn):
            hooks[(h, g)].append(fn)

        mk = lambda fn, *a: (lambda: fn(*a))
        for g in range(6):                       # pair 0 head 0: V ic0 jit
            at(0, g, mk(v_unit, 2 * g + 4, 0))
            at(0, g, mk(v_unit, 2 * g + 5, 0))
        at(0, 0, swap_wq_for_wo)
        at(7, 5, mk(v_unit, 0, 1))
        at(7, 5, mk(v_unit, 1, 1))
        at(7, 6, mk(v_unit, 2, 1))
        at(7, 6, mk(v_unit, 3, 1))
        for g in range(6):                       # pair 4 head 8: V ic1 jit
            at(8, g, mk(v_unit, 2 * g + 4, 1))
            at(8, g, mk(v_unit, 2 * g + 5, 1))
        # out-proj p split across both heads of pair (p//2 + 1) so every
        # head from h2 on carries ~0.85us of PE filler
        for p in range(7):
            for g in range(8):
                qb, dc = divmod(g, 2)
                h = 2 * p + 2 + g // 4
                at(h, 4 * (g // 4 == 0) + g % 4, mk(o_unit, p, qb, dc))

        # ---------------- attention heads ----------------
        def head_attn(h, tail_cb=None):
            hp, hh = h // 2, h % 2
            op = psb.tile([128, QR], f32, tag="op", bufs=2, name="op")
            for g in range(KB // 2):
                sp = psb.tile([128, 2, 512], f32, tag="sp", bufs=2,
                              name="sp")
                for j in range(2):
                    kb = 2 * g + j
                    nc.tensor.matmul(
                        out=sp[:, j, :],
                        lhsT=KT_sb[:, hp, kb * 128:(kb + 1) * 128],
                        rhs=QT_z[:, hp, hh, :],
                        start=True, stop=True)
                expS = pes.tile([128, 2, 512], bf16, tag="es", bufs=2,
                                name="expS")
                nc.scalar.activation(out=expS, in_=sp, func=Exp, scale=SCALE)
                for j in range(2):
                    kb = 2 * g + j
                    nc.tensor.matmul(
                        out=op[0:65, :],
                        lhsT=V_sb[:, kb, h, :],
                        rhs=expS[:, j, :],
                        start=(kb == 0), stop=(kb == KB - 1))
                for fn in hooks.get((h, g), ()):
                    fn()
            recip = psm.tile([1, QR], f32, tag="recip", bufs=2, name="recip")
            nc.vector.reciprocal(out=recip, in_=op[64:65, :])
            rbs = psm.tile([64, QR], f32, tag="rbs", bufs=1, name="rbs")
            nc.gpsimd.partition_broadcast(out_ap=rbs, in_ap=recip)
            nc.vector.tensor_mul(
                OT_sb[hh * 64:(hh + 1) * 64, hp, :], op[0:64, :], rbs)
            if tail_cb is not None:
                for qb in range(QB):
                    tail_cb(qb)

        # prologue: own-key K units + pipelined gathers, Q under DMA wait
        for ib in range(IB):
            k_unit(ib)
            if ib % 2 == 1:
                k_gather(ib // 2)
            if ib < 6:
                q_unit(ib)
        for kb in range(4):
            v_unit(kb, 0)
        q_unit(6)
        q_unit(7)
        for h in range(H):
            head_attn(h)
        for qb in range(QB):                     # pair 7 combined, K=128
            for dc in range(2):
                o_unit(7, qb, dc)
        wo_state["pool"].release()

    nc.compile()
    return nc


def _get_nc():
    if "nc" not in _cached:
        _cached["nc"] = _build()
    return _cached["nc"]


def kernel(queries, Wq, Wkv, Wo, bo, _trace=False):
    import ml_dtypes
    from concourse.bass_utils import run_bass_kernel_spmd

    queries = np.asarray(queries, dtype=np.float32)
    bo = np.asarray(bo, dtype=np.float32)
    cast = lambda a: np.ascontiguousarray(a).astype(ml_dtypes.bfloat16)
    Wq_c = cast(Wq)
    Wk_c = cast(np.asarray(Wkv)[:, 0:INNER])
    Wv_c = cast(np.asarray(Wkv)[:, INNER:2 * INNER])
    Wo_c = cast(Wo)

    nc = _get_nc()

    in_maps = []
    xTb = [cast(queries[g].T) for g in range(B)]         # [DM, N] unrotated
    for c in range(NCORES):
        g, r = c // 4, c % 4
        xq = np.ascontiguousarray(xTb[g][:, r * QR:(r + 1) * QR])
        in_maps.append({"xT": xTb[g], "xq": xq, "Wq": Wq_c, "Wk": Wk_c,
                        "Wv": Wv_c, "Wo": Wo_c, "bo": bo})

    res = run_bass_kernel_spmd(nc, in_maps, list(range(NCORES)),
                               trace=_trace)
    out = np.empty((B, N, DM), dtype=np.float32)
    for c in range(NCORES):
        g, r = c // 4, c % 4
        out[g, r * QR:(r + 1) * QR, :] = res.results[c]["out"]
    if _trace:
        return out, res
    return out


if __name__ == "__main__":
    rng = np.random.default_rng(0)
    q = rng.standard_normal((B, N, DM), dtype=np.float32)
    s = 0.02
    inputs = dict(
        queries=q,
        Wq=(rng.standard_normal((DM, INNER), dtype=np.float32) * s),
        Wkv=(rng.standard_normal((DM, 2 * INNER), dtype=np.float32) * s),
        Wo=(rng.standard_normal((INNER, DM), dtype=np.float32) * s),
        bo=(rng.standard_normal((DM,), dtype=np.float32) * s),
    )
    out = kernel(**inputs)
    print("kernel ran, out shape", out.shape)


# revision 50
# speedup vs baseline: 1.0558x; 1.0558x over previous
"""Trainium2 Bass kernel for 16-head self-attention (b=2, n=2048, dm=1024, dh=64).

Sharding: each of 8 cores owns (batch g = c//4, sequence block r = c%4).
A core computes K,V for its batch's FULL sequence (replicated across the 4
cores of that batch -- avoids cross-core collectives), attention for all 16
heads restricted to its 512 query rows, and the output projection for those
rows.  Per-core outputs are disjoint [512, 1024] slices of the final
[2, 2048, 1024]; the host concatenates.

Host passes x^T (dm-major) ROTATED by the core's row offset, so every core's
query slice is columns 0:512 of its own x^T -- the SPMD program is identical
across cores.  Attention is permutation-invariant over keys, so the rotation
does not change the result.

v3: single hand-woven schedule.  The scalar engine's exp stream (16 heads x
8 instructions x ~1.1us over [128, 2*512] PSUM pairs) is the attention-phase
floor, and the PE drops to its mid p-state (1.2 GHz, 2x slower matmuls)
whenever it idles waiting on it.  So instead of phased projections followed
by attention, only Q(ib=0)+K(ib=0) run up front; attention pair 0 starts
~15us in, and ALL remaining work -- Q(1..7), K(1..7) per 512-key chunk,
V units (one [128 keys x 512 inner-cols] block each), and the per-pair
rank-128 output-projection updates -- is interleaved between the S/exp/AV
groups as PE filler with just-in-time deadlines (K(p) before pair p,
V(kb,ic) blocks two kb ahead of the consuming AV, Wq freed after Q(7) to
make room for Wo).  This keeps ACT busy from the start, the PE dense (and
hence at full clock), and leaves only the last pair's softmax tail exposed.

Other structure (from v2): all-bf16 operands; K^T and V entirely in SBUF;
V stored [128, kb, head, 65] with a ones column so the AV matmul emits the
softmax denominator for free; exact DVE reciprocal (approx_fast
miscompiles on this toolchain; ACT Ln/Exp reciprocal costs two 1.3us table
reloads); gpsimd partition_broadcast for the 1/denom cross-partition
broadcast (gpsimd cannot touch PSUM, so the normalize multiply and the
accumulator adds stay on DVE).
"""

import sys

for _p in ("/opt/trn_rl_repo", "/root/.axon_site/_ro/trn_rl_repo"):
    if _p not in sys.path:
        sys.path.append(_p)

import numpy as np

B = 2
N = 2048
DM = 1024
H = 16
DH = 64
INNER = H * DH  # 1024
NCORES = 8
QR = 512  # query rows per core
SCALE = DH ** -0.5

_cached = {}


def _build():
    import contextlib
    import concourse.bacc as bacc
    import concourse.tile as tile
    import concourse.mybir as mybir

    f32 = mybir.dt.float32
    bf16 = mybir.dt.bfloat16
    Exp = mybir.ActivationFunctionType.Exp

    nc = bacc.Bacc("TRN2", target_bir_lowering=False, debug=False,
                   enable_asserts=False)

    xT_d = nc.dram_tensor("xT", [DM, N], bf16, kind="ExternalInput").ap()
    Wq_d = nc.dram_tensor("Wq", [DM, INNER], bf16, kind="ExternalInput").ap()
    Wk_d = nc.dram_tensor("Wk", [DM, INNER], bf16, kind="ExternalInput").ap()
    Wv_d = nc.dram_tensor("Wv", [DM, INNER], bf16, kind="ExternalInput").ap()
    Wo_d = nc.dram_tensor("Wo", [INNER, DM], bf16, kind="ExternalInput").ap()
    bo_d = nc.dram_tensor("bo", [DM], f32, kind="ExternalInput").ap()
    out_d = nc.dram_tensor("out", [QR, DM], f32, kind="ExternalOutput").ap()

    A = DM // 128      # 8 dm blocks
    IB = INNER // 128  # 8 inner blocks
    KB = N // 128      # 16 key blocks
    KC = N // 512      # 4 key chunks
    QB = QR // 128     # 4 query blocks

    with tile.TileContext(nc) as tc, \
         nc.allow_low_precision(reason="bf16 matmul pipeline, validated e2e"), \
         contextlib.ExitStack() as ctx:
        persist = ctx.enter_context(tc.tile_pool(name="persist", bufs=1))
        xT_sb = persist.tile([128, A, N], bf16)
        QT_z = persist.tile([128, IB, 2, QR], bf16)
        KT_sb = persist.tile([128, IB, N], bf16)    # K^T [inner, keys]
        V_sb = persist.tile([128, KB, H, 65], bf16)  # V + ones col per head
        OT_sb = persist.tile([128, IB, QR], bf16)   # O^T  [inner, q]
        out_acc = persist.tile([128, QB, DM], f32)  # out rows accumulator
        bo_sb = persist.tile([128, DM], f32)
        onef = persist.tile([128, 1], f32)
        zerof = persist.tile([128, 1], f32)

        nc.vector.memset(onef, 1.0)
        nc.vector.memset(zerof, 0.0)
        nc.vector.tensor_copy(
            out=QT_z[:, :, :, :],
            in_=zerof.unsqueeze(1).unsqueeze(1).to_broadcast(
                [128, IB, 2, QR]))
        nc.vector.tensor_copy(
            out=V_sb[:, :, :, 64:65],
            in_=onef.unsqueeze(1).unsqueeze(1).to_broadcast([128, KB, H, 1]))
        nc.gpsimd.dma_start(
            out=bo_sb, in_=bo_d.unsqueeze(0).to_broadcast([128, DM]))

        xT_r = xT_d.rearrange("(a p) n -> a p n", p=128)
        Wq_r = Wq_d.rearrange("(a p) i -> a p i", p=128)
        Wk_r = Wk_d.rearrange("(a p) i -> a p i", p=128)
        Wv_r = Wv_d.rearrange("(a p) i -> a p i", p=128)
        Wo_r = Wo_d.rearrange("(ib p) d -> ib p d", p=128)

        wpool = ctx.enter_context(tc.tile_pool(name="wqk", bufs=1))
        Wk_sb = wpool.tile([128, A, INNER], bf16)
        Wv_sb = wpool.tile([128, A, INNER], bf16)
        pes = ctx.enter_context(tc.tile_pool(name="pb_es", bufs=1))
        psm = ctx.enter_context(tc.tile_pool(name="pb_sm", bufs=1))
        psb = ctx.enter_context(tc.tile_pool(name="pb_ps", bufs=1,
                                             space="PSUM"))
        # Wq lives in its own pool, allocated last, so releasing it after
        # Q(7) frees the top of the SBUF stack for Wo.
        wq_pool = tc.alloc_tile_pool(name="wq", bufs=1)
        Wq_sb = wq_pool.tile([128, A, INNER], bf16)

        # DMA order = need order: the core's own query columns of x^T
        # (for Q(0)), Wq, then full x^T rows, Wk, Wv.
        for a in range(A):
            nc.sync.dma_start(out=xT_sb[:, a, 0:QR], in_=xT_r[a][:, 0:QR])
        for a in range(A):
            nc.sync.dma_start(out=Wq_sb[:, a, :], in_=Wq_r[a])
        for a in range(A):
            nc.sync.dma_start(out=xT_sb[:, a, QR:N], in_=xT_r[a][:, QR:N])
        for a in range(A):
            nc.sync.dma_start(out=Wk_sb[:, a, :], in_=Wk_r[a])
        for a in range(A):
            nc.sync.dma_start(out=Wv_sb[:, a, :], in_=Wv_r[a])

        wo_state = {}

        # ---------------- fill units ----------------
        def q_unit(ib):
            qp = psb.tile([128, QR], f32, tag="proj", bufs=2, name="qp")
            for a in range(A):
                nc.tensor.matmul(
                    out=qp,
                    lhsT=Wq_sb[:, a, ib * 128:(ib + 1) * 128],
                    rhs=xT_sb[:, a, 0:QR],
                    start=(a == 0), stop=(a == A - 1))
            nc.vector.tensor_copy(out=QT_z[0:64, ib, 0, :], in_=qp[0:64, :])
            nc.vector.tensor_copy(out=QT_z[64:128, ib, 1, :],
                                  in_=qp[64:128, :])

        def k_unit(ib, kc):
            kp = psb.tile([128, 512], f32, tag="proj", bufs=2, name="kp")
            for a in range(A):
                nc.tensor.matmul(
                    out=kp,
                    lhsT=Wk_sb[:, a, ib * 128:(ib + 1) * 128],
                    rhs=xT_sb[:, a, kc * 512:(kc + 1) * 512],
                    start=(a == 0), stop=(a == A - 1))
            nc.vector.tensor_copy(
                out=KT_sb[:, ib, kc * 512:(kc + 1) * 512], in_=kp)

        def v_unit(kb, ic):
            vp = psb.tile([128, 512], f32, tag="proj", bufs=2, name="vp")
            for a in range(A):
                nc.tensor.matmul(
                    out=vp,
                    lhsT=xT_sb[:, a, kb * 128:(kb + 1) * 128],
                    rhs=Wv_sb[:, a, ic * 512:(ic + 1) * 512],
                    start=(a == 0), stop=(a == A - 1))
            nc.vector.tensor_copy(
                out=V_sb[:, kb, ic * 8:(ic + 1) * 8, 0:64], in_=vp)

        def swap_wq_for_wo():
            wq_pool.release()
            wo_pool = tc.alloc_tile_pool(name="wo", bufs=1)
            Wo_sb = wo_pool.tile([128, IB, DM], bf16)
            for ib in range(IB):
                nc.sync.dma_start(out=Wo_sb[:, ib, :], in_=Wo_r[ib])
            wo_state["Wo_sb"] = Wo_sb
            wo_state["pool"] = wo_pool

        def o_unit(hp, qb, dc):
            Wo_sb = wo_state["Wo_sb"]
            outp = psb.tile([128, 512], f32, tag="proj", bufs=2, name="outp")
            nc.tensor.matmul(
                out=outp,
                lhsT=OT_sb[:, hp, qb * 128:(qb + 1) * 128],
                rhs=Wo_sb[:, hp, dc * 512:(dc + 1) * 512],
                start=True, stop=True)
            dst = out_acc[:, qb, dc * 512:(dc + 1) * 512]
            if hp == 0:
                nc.vector.tensor_add(
                    dst, outp, bo_sb[:, dc * 512:(dc + 1) * 512])
            else:
                nc.vector.tensor_add(dst, outp, dst)
            if hp == IB - 1:
                nc.sync.dma_start(
                    out=out_d[qb * 128:(qb + 1) * 128,
                              dc * 512:(dc + 1) * 512],
                    in_=dst)

        def o_tail_chunk(hh, qb, final):
            # pair-7 per-head rank-64 update for one query block
            Wo_sb = wo_state["Wo_sb"]
            for dc in range(2):
                outp = psb.tile([128, 512], f32, tag="proj", bufs=2,
                                name="outp")
                nc.tensor.matmul(
                    out=outp,
                    lhsT=OT_sb[hh * 64:(hh + 1) * 64, 7,
                               qb * 128:(qb + 1) * 128],
                    rhs=Wo_sb[hh * 64:(hh + 1) * 64, 7,
                              dc * 512:(dc + 1) * 512],
                    start=True, stop=True)
                dst = out_acc[:, qb, dc * 512:(dc + 1) * 512]
                nc.vector.tensor_add(dst, outp, dst)
                if final:
                    nc.sync.dma_start(
                        out=out_d[qb * 128:(qb + 1) * 128,
                                  dc * 512:(dc + 1) * 512],
                        in_=dst)

        # ---------------- fill schedule ----------------
        # hooks[(h, g)] -> list of zero-arg fill closures emitted after
        # S/exp/AV group g of head h.  Deadlines honored by construction:
        #   K(p), Q(p) complete before pair p;  V(kb, ic) lands >= 2 kb
        #   ahead of the first consuming AV (head 0 for ic0, head 8 for
        #   ic1);  out-proj updates only need OT rows finished a pair ago.
        from collections import defaultdict
        hooks = defaultdict(list)

        def at(h, g, fn):
            hooks[(h, g)].append(fn)

        mk = lambda fn, *a: (lambda: fn(*a))
        for g in range(6):                       # pair 0 head 0: V ic0 jit
            at(0, g, mk(v_unit, 2 * g + 4, 0))
            at(0, g, mk(v_unit, 2 * g + 5, 0))
        for kc in range(KC):                     # pair 0 head 1
            at(1, 1 + kc, mk(k_unit, 1, kc))
        for kc in range(KC):                     # pair 1
            at(2, 1 + kc, mk(k_unit, 2, kc))
        for kc in range(KC):
            at(3, 1 + kc, mk(k_unit, 3, kc))
        for kc in range(KC):                     # pair 2
            at(4, 1 + kc, mk(k_unit, 4, kc))
        for kc in range(KC):
            at(5, 1 + kc, mk(k_unit, 5, kc))
        at(5, 7, swap_wq_for_wo)
        at(7, 5, mk(v_unit, 0, 1))               # pair 3: V ic1 head start
        at(7, 5, mk(v_unit, 1, 1))
        at(7, 6, mk(v_unit, 2, 1))
        at(7, 6, mk(v_unit, 3, 1))
        for g in range(6):                       # pair 4 head 8: V ic1 jit
            at(8, g, mk(v_unit, 2 * g + 4, 1))
            at(8, g, mk(v_unit, 2 * g + 5, 1))
        for kc in range(KC):                     # K(6) late: feeds pair 4
            at(9, 1 + kc, mk(k_unit, 6, kc))
        for kc in range(2):                      # K(7) split over pair 6
            at(12, 1 + kc, mk(k_unit, 7, kc))
            at(13, 1 + kc, mk(k_unit, 7, 2 + kc))
        for i, h in enumerate((10, 11, 12, 13, 14, 15)):  # out-proj 0-5
            for g in range(8):
                qb, dc = divmod(g, 2)
                at(h, g, mk(o_unit, i, qb, dc))

        # ---------------- attention heads ----------------
        def head_attn(h, tail_cb=None):
            hp, hh = h // 2, h % 2
            op = psb.tile([128, QR], f32, tag="op", bufs=2, name="op")
            for g in range(KB // 2):
                sp = psb.tile([128, 2, 512], f32, tag="sp", bufs=2,
                              name="sp")
                for j in range(2):
                    kb = 2 * g + j
                    nc.tensor.matmul(
                        out=sp[:, j, :],
                        lhsT=KT_sb[:, hp, kb * 128:(kb + 1) * 128],
                        rhs=QT_z[:, hp, hh, :],
                        start=True, stop=True)
                expS = pes.tile([128, 2, 512], bf16, tag="es", bufs=3,
                                name="expS")
                nc.scalar.activation(out=expS, in_=sp, func=Exp, scale=SCALE)
                for j in range(2):
                    kb = 2 * g + j
                    nc.tensor.matmul(
                        out=op[0:65, :],
                        lhsT=V_sb[:, kb, h, :],
                        rhs=expS[:, j, :],
                        start=(kb == 0), stop=(kb == KB - 1))
                for fn in hooks.get((h, g), ()):
                    fn()
            recip = psm.tile([1, QR], f32, tag="recip", bufs=2, name="recip")
            nc.vector.reciprocal(out=recip, in_=op[64:65, :])
            rbs = psm.tile([64, QR], f32, tag="rbs", bufs=2, name="rbs")
            nc.gpsimd.partition_broadcast(out_ap=rbs, in_ap=recip)
            nc.vector.tensor_mul(
                OT_sb[hh * 64:(hh + 1) * 64, hp, :], op[0:64, :], rbs)
            if tail_cb is not None:
                for qb in range(QB):
                    tail_cb(qb)

        # prologue: all Q units fill the x^T/Wk/Wv DMA wait, then K(0) and
        # the first V ic0 blocks
        for ib in range(IB):
            q_unit(ib)
        for kc in range(KC):
            k_unit(0, kc)
        for kb in range(4):
            v_unit(kb, 0)
        for h in range(H):
            head_attn(h)
        for qb in range(QB):                     # pair 6: fills the PE while
            for dc in range(2):                  # h15's softmax tail drains
                o_unit(6, qb, dc)
        for qb in range(QB):                     # pair 7 combined, K=128
            for dc in range(2):
                o_unit(7, qb, dc)
        wo_state["pool"].release()

    nc.compile()
    return nc


def _get_nc():
    if "nc" not in _cached:
        _cached["nc"] = _build()
    return _cached["nc"]


def kernel(queries, Wq, Wkv, Wo, bo, _trace=False):
    import ml_dtypes
    from concourse.bass_utils import run_bass_kernel_spmd

    queries = np.asarray(queries, dtype=np.float32)
    bo = np.asarray(bo, dtype=np.float32)
    cast = lambda a: np.ascontiguousarray(a).astype(ml_dtypes.bfloat16)
    Wq_c = cast(Wq)
    Wk_c = cast(np.asarray(Wkv)[:, 0:INNER])
    Wv_c = cast(np.asarray(Wkv)[:, INNER:2 * INNER])
    Wo_c = cast(Wo)

    nc = _get_nc()

    in_maps = []
    for c in range(NCORES):
        g, r = c // 4, c % 4
        xT = np.ascontiguousarray(queries[g].T)          # [DM, N]
        xT = cast(np.roll(xT, -r * QR, axis=1))
        in_maps.append({"xT": xT, "Wq": Wq_c, "Wk": Wk_c, "Wv": Wv_c,
                        "Wo": Wo_c, "bo": bo})

    res = run_bass_kernel_spmd(nc, in_maps, list(range(NCORES)),
                               trace=_trace)
    out = np.empty((B, N, DM), dtype=np.float32)
    for c in range(NCORES):
        g, r = c // 4, c % 4
        out[g, r * QR:(r + 1) * QR, :] = res.results[c]["out"]
    if _trace:
        return out, res
    return out


if __name__ == "__main__":
    rng = np.random.default_rng(0)
    q = rng.standard_normal((B, N, DM), dtype=np.float32)
    s = 0.02
    inputs = dict(
        queries=q,
        Wq=(rng.standard_normal((DM, INNER), dtype=np.float32) * s),
        Wkv=(rng.standard_normal((DM, 2 * INNER), dtype=np.float32) * s),
        Wo=(rng.standard_normal((INNER, DM), dtype=np.float32) * s),
        bo=(rng.standard_normal((DM,), dtype=np.float32) * s),
    )
    out = kernel(**inputs)
    print("kernel ran, out shape", out.shape)


# revision 51
# speedup vs baseline: 1.1061x; 1.0476x over previous
"""Trainium2 Bass kernel for 16-head self-attention (b=2, n=2048, dm=1024, dh=64).

Sharding: each of 8 cores owns (batch g = c//4, sequence block r = c%4).
A core computes K,V for its batch's FULL sequence (replicated across the 4
cores of that batch -- avoids cross-core collectives), attention for all 16
heads restricted to its 512 query rows, and the output projection for those
rows.  Per-core outputs are disjoint [512, 1024] slices of the final
[2, 2048, 1024]; the host concatenates.

Host passes x^T (dm-major) ROTATED by the core's row offset, so every core's
query slice is columns 0:512 of its own x^T -- the SPMD program is identical
across cores.  Attention is permutation-invariant over keys, so the rotation
does not change the result.

v3: single hand-woven schedule.  The scalar engine's exp stream (16 heads x
8 instructions x ~1.1us over [128, 2*512] PSUM pairs) is the attention-phase
floor, and the PE drops to its mid p-state (1.2 GHz, 2x slower matmuls)
whenever it idles waiting on it.  So instead of phased projections followed
by attention, only Q(ib=0)+K(ib=0) run up front; attention pair 0 starts
~15us in, and ALL remaining work -- Q(1..7), K(1..7) per 512-key chunk,
V units (one [128 keys x 512 inner-cols] block each), and the per-pair
rank-128 output-projection updates -- is interleaved between the S/exp/AV
groups as PE filler with just-in-time deadlines (K(p) before pair p,
V(kb,ic) blocks two kb ahead of the consuming AV, Wq freed after Q(7) to
make room for Wo).  This keeps ACT busy from the start, the PE dense (and
hence at full clock), and leaves only the last pair's softmax tail exposed.

Other structure (from v2): all-bf16 operands; K^T and V entirely in SBUF;
V stored [128, kb, head, 65] with a ones column so the AV matmul emits the
softmax denominator for free; exact DVE reciprocal (approx_fast
miscompiles on this toolchain; ACT Ln/Exp reciprocal costs two 1.3us table
reloads); gpsimd partition_broadcast for the 1/denom cross-partition
broadcast (gpsimd cannot touch PSUM, so the normalize multiply and the
accumulator adds stay on DVE).
"""

import sys

for _p in ("/opt/trn_rl_repo", "/root/.axon_site/_ro/trn_rl_repo"):
    if _p not in sys.path:
        sys.path.append(_p)

import numpy as np

B = 2
N = 2048
DM = 1024
H = 16
DH = 64
INNER = H * DH  # 1024
NCORES = 8
QR = 512  # query rows per core
SCALE = DH ** -0.5

_cached = {}


def _build():
    import contextlib
    import concourse.bacc as bacc
    import concourse.tile as tile
    import concourse.mybir as mybir

    f32 = mybir.dt.float32
    bf16 = mybir.dt.bfloat16
    fp8 = mybir.dt.float8e4
    Exp = mybir.ActivationFunctionType.Exp

    nc = bacc.Bacc("TRN2", target_bir_lowering=False, debug=False,
                   enable_asserts=False)

    xT_d = nc.dram_tensor("xT", [DM, N], bf16, kind="ExternalInput").ap()
    Wq_d = nc.dram_tensor("Wq", [DM, INNER], bf16, kind="ExternalInput").ap()
    Wk_d = nc.dram_tensor("Wk", [DM, INNER], bf16, kind="ExternalInput").ap()
    Wv_d = nc.dram_tensor("Wv", [DM, INNER], bf16, kind="ExternalInput").ap()
    Wo_d = nc.dram_tensor("Wo", [INNER, DM], bf16, kind="ExternalInput").ap()
    bo_d = nc.dram_tensor("bo", [DM], f32, kind="ExternalInput").ap()
    out_d = nc.dram_tensor("out", [QR, DM], f32, kind="ExternalOutput").ap()

    A = DM // 128      # 8 dm blocks
    IB = INNER // 128  # 8 inner blocks
    KB = N // 128      # 16 key blocks
    KC = N // 512      # 4 key chunks
    QB = QR // 128     # 4 query blocks

    with tile.TileContext(nc) as tc, \
         nc.allow_low_precision(reason="bf16 matmul pipeline, validated e2e"), \
         contextlib.ExitStack() as ctx:
        persist = ctx.enter_context(tc.tile_pool(name="persist", bufs=1))
        xT_sb = persist.tile([128, A, N], bf16)
        QT_z = persist.tile([128, IB, 2, 2, QR], fp8)
        KT_sb = persist.tile([128, IB, 2, N], fp8)  # K^T + zero subtile
        V_sb = persist.tile([128, KB, H, 65], bf16)  # V + ones col per head
        OT_sb = persist.tile([128, IB, QR], bf16)   # O^T  [inner, q]
        out_acc = persist.tile([128, QB, DM], f32)  # out rows accumulator
        bo_sb = persist.tile([128, DM], f32)
        onef = persist.tile([128, 1], f32)
        zerof = persist.tile([128, 1], f32)

        nc.vector.memset(onef, 1.0)
        nc.vector.memset(zerof, 0.0)
        nc.vector.tensor_copy(
            out=QT_z[:, :, :, :, :],
            in_=zerof.unsqueeze(1).unsqueeze(1).unsqueeze(1).to_broadcast(
                [128, IB, 2, 2, QR]))
        nc.vector.tensor_copy(
            out=KT_sb[:, :, 1, :],
            in_=zerof.unsqueeze(1).to_broadcast([128, IB, N]))
        nc.vector.tensor_copy(
            out=V_sb[:, :, :, 64:65],
            in_=onef.unsqueeze(1).unsqueeze(1).to_broadcast([128, KB, H, 1]))
        nc.gpsimd.dma_start(
            out=bo_sb, in_=bo_d.unsqueeze(0).to_broadcast([128, DM]))

        xT_r = xT_d.rearrange("(a p) n -> a p n", p=128)
        Wq_r = Wq_d.rearrange("(a p) i -> a p i", p=128)
        Wk_r = Wk_d.rearrange("(a p) i -> a p i", p=128)
        Wv_r = Wv_d.rearrange("(a p) i -> a p i", p=128)
        Wo_r = Wo_d.rearrange("(ib p) d -> ib p d", p=128)

        wpool = ctx.enter_context(tc.tile_pool(name="wqk", bufs=1))
        Wk_sb = wpool.tile([128, A, INNER], bf16)
        Wv_sb = wpool.tile([128, A, INNER], bf16)
        pes = ctx.enter_context(tc.tile_pool(name="pb_es", bufs=1))
        psm = ctx.enter_context(tc.tile_pool(name="pb_sm", bufs=1))
        psb = ctx.enter_context(tc.tile_pool(name="pb_ps", bufs=1,
                                             space="PSUM"))
        # Wq lives in its own pool, allocated last, so releasing it after
        # Q(7) frees the top of the SBUF stack for Wo.
        wq_pool = tc.alloc_tile_pool(name="wq", bufs=1)
        Wq_sb = wq_pool.tile([128, A, INNER], bf16)

        # DMA order = need order: the core's own query columns of x^T
        # (for Q(0)), Wq, then full x^T rows, Wk, Wv.
        for a in range(A):
            nc.sync.dma_start(out=xT_sb[:, a, 0:QR], in_=xT_r[a][:, 0:QR])
        for a in range(A):
            nc.sync.dma_start(out=Wq_sb[:, a, :], in_=Wq_r[a])
        for a in range(A):
            nc.sync.dma_start(out=xT_sb[:, a, QR:N], in_=xT_r[a][:, QR:N])
        for a in range(A):
            nc.sync.dma_start(out=Wk_sb[:, a, :], in_=Wk_r[a])
        for a in range(A):
            nc.sync.dma_start(out=Wv_sb[:, a, :], in_=Wv_r[a])

        wo_state = {}

        # ---------------- fill units ----------------
        def q_unit(ib):
            qp = psb.tile([128, QR], f32, tag="proj", bufs=2, name="qp")
            for a in range(A):
                nc.tensor.matmul(
                    out=qp,
                    lhsT=Wq_sb[:, a, ib * 128:(ib + 1) * 128],
                    rhs=xT_sb[:, a, 0:QR],
                    start=(a == 0), stop=(a == A - 1))
            nc.vector.tensor_copy(out=QT_z[0:64, ib, 0, 0, :],
                                  in_=qp[0:64, :])
            nc.vector.tensor_copy(out=QT_z[64:128, ib, 1, 0, :],
                                  in_=qp[64:128, :])

        def k_unit(ib, kc):
            kp = psb.tile([128, 512], f32, tag="proj", bufs=2, name="kp")
            for a in range(A):
                nc.tensor.matmul(
                    out=kp,
                    lhsT=Wk_sb[:, a, ib * 128:(ib + 1) * 128],
                    rhs=xT_sb[:, a, kc * 512:(kc + 1) * 512],
                    start=(a == 0), stop=(a == A - 1))
            nc.vector.tensor_copy(
                out=KT_sb[:, ib, 0, kc * 512:(kc + 1) * 512], in_=kp)

        def v_unit(kb, ic):
            vp = psb.tile([128, 512], f32, tag="proj", bufs=2, name="vp")
            for a in range(A):
                nc.tensor.matmul(
                    out=vp,
                    lhsT=xT_sb[:, a, kb * 128:(kb + 1) * 128],
                    rhs=Wv_sb[:, a, ic * 512:(ic + 1) * 512],
                    start=(a == 0), stop=(a == A - 1))
            nc.vector.tensor_copy(
                out=V_sb[:, kb, ic * 8:(ic + 1) * 8, 0:64], in_=vp)

        def swap_wq_for_wo():
            wq_pool.release()
            wo_pool = tc.alloc_tile_pool(name="wo", bufs=1)
            Wo_sb = wo_pool.tile([128, IB, DM], bf16)
            for ib in range(IB):
                nc.sync.dma_start(out=Wo_sb[:, ib, :], in_=Wo_r[ib])
            wo_state["Wo_sb"] = Wo_sb
            wo_state["pool"] = wo_pool

        def o_unit(hp, qb, dc):
            Wo_sb = wo_state["Wo_sb"]
            outp = psb.tile([128, 512], f32, tag="proj", bufs=2, name="outp")
            nc.tensor.matmul(
                out=outp,
                lhsT=OT_sb[:, hp, qb * 128:(qb + 1) * 128],
                rhs=Wo_sb[:, hp, dc * 512:(dc + 1) * 512],
                start=True, stop=True)
            dst = out_acc[:, qb, dc * 512:(dc + 1) * 512]
            if hp == 0:
                nc.vector.tensor_add(
                    dst, outp, bo_sb[:, dc * 512:(dc + 1) * 512])
            else:
                nc.vector.tensor_add(dst, outp, dst)
            if hp == IB - 1:
                nc.sync.dma_start(
                    out=out_d[qb * 128:(qb + 1) * 128,
                              dc * 512:(dc + 1) * 512],
                    in_=dst)

        def o_tail_chunk(hh, qb, final):
            # pair-7 per-head rank-64 update for one query block
            Wo_sb = wo_state["Wo_sb"]
            for dc in range(2):
                outp = psb.tile([128, 512], f32, tag="proj", bufs=2,
                                name="outp")
                nc.tensor.matmul(
                    out=outp,
                    lhsT=OT_sb[hh * 64:(hh + 1) * 64, 7,
                               qb * 128:(qb + 1) * 128],
                    rhs=Wo_sb[hh * 64:(hh + 1) * 64, 7,
                              dc * 512:(dc + 1) * 512],
                    start=True, stop=True)
                dst = out_acc[:, qb, dc * 512:(dc + 1) * 512]
                nc.vector.tensor_add(dst, outp, dst)
                if final:
                    nc.sync.dma_start(
                        out=out_d[qb * 128:(qb + 1) * 128,
                                  dc * 512:(dc + 1) * 512],
                        in_=dst)

        # ---------------- fill schedule ----------------
        # hooks[(h, g)] -> list of zero-arg fill closures emitted after
        # S/exp/AV group g of head h.  Deadlines honored by construction:
        #   K(p), Q(p) complete before pair p;  V(kb, ic) lands >= 2 kb
        #   ahead of the first consuming AV (head 0 for ic0, head 8 for
        #   ic1);  out-proj updates only need OT rows finished a pair ago.
        from collections import defaultdict
        hooks = defaultdict(list)

        def at(h, g, fn):
            hooks[(h, g)].append(fn)

        mk = lambda fn, *a: (lambda: fn(*a))
        for g in range(6):                       # pair 0 head 0: V ic0 jit
            at(0, g, mk(v_unit, 2 * g + 4, 0))
            at(0, g, mk(v_unit, 2 * g + 5, 0))
        for kc in range(KC):                     # pair 0 head 1
            at(1, 1 + kc, mk(k_unit, 1, kc))
        for kc in range(KC):                     # pair 1
            at(2, 1 + kc, mk(k_unit, 2, kc))
        for kc in range(KC):
            at(3, 1 + kc, mk(k_unit, 3, kc))
        for kc in range(KC):                     # pair 2
            at(4, 1 + kc, mk(k_unit, 4, kc))
        for kc in range(KC):
            at(5, 1 + kc, mk(k_unit, 5, kc))
        at(5, 7, swap_wq_for_wo)
        at(7, 5, mk(v_unit, 0, 1))               # pair 3: V ic1 head start
        at(7, 5, mk(v_unit, 1, 1))
        at(7, 6, mk(v_unit, 2, 1))
        at(7, 6, mk(v_unit, 3, 1))
        for g in range(6):                       # pair 4 head 8: V ic1 jit
            at(8, g, mk(v_unit, 2 * g + 4, 1))
            at(8, g, mk(v_unit, 2 * g + 5, 1))
        for kc in range(KC):                     # K(6) late: feeds pair 4
            at(9, 1 + kc, mk(k_unit, 6, kc))
        for kc in range(2):                      # K(7) split over pair 6
            at(12, 1 + kc, mk(k_unit, 7, kc))
            at(13, 1 + kc, mk(k_unit, 7, 2 + kc))
        for i, h in enumerate((10, 11, 12, 13, 14, 15)):  # out-proj 0-5
            for g in range(8):
                qb, dc = divmod(g, 2)
                at(h, g, mk(o_unit, i, qb, dc))

        # ---------------- attention heads ----------------
        def head_attn(h, tail_cb=None):
            hp, hh = h // 2, h % 2
            op = psb.tile([128, QR], f32, tag="op", bufs=2, name="op")
            for g in range(KB // 2):
                sp = psb.tile([128, 2, 512], f32, tag="sp", bufs=2,
                              name="sp")
                for j in range(2):
                    kb = 2 * g + j
                    nc.tensor.matmul(
                        out=sp[:, j, :],
                        lhsT=KT_sb[:, hp, :, kb * 128:(kb + 1) * 128],
                        rhs=QT_z[:, hp, hh, :, :],
                        start=True, stop=True,
                        perf_mode=mybir.MatmulPerfMode.DoubleRow)
                expS = pes.tile([128, 2, 512], bf16, tag="es", bufs=3,
                                name="expS")
                nc.scalar.activation(out=expS, in_=sp, func=Exp, scale=SCALE)
                for j in range(2):
                    kb = 2 * g + j
                    nc.tensor.matmul(
                        out=op[0:65, :],
                        lhsT=V_sb[:, kb, h, :],
                        rhs=expS[:, j, :],
                        start=(kb == 0), stop=(kb == KB - 1))
                for fn in hooks.get((h, g), ()):
                    fn()
            recip = psm.tile([1, QR], f32, tag="recip", bufs=2, name="recip")
            nc.vector.reciprocal(out=recip, in_=op[64:65, :])
            rbs = psm.tile([64, QR], f32, tag="rbs", bufs=2, name="rbs")
            nc.gpsimd.partition_broadcast(out_ap=rbs, in_ap=recip)
            nc.vector.tensor_mul(
                OT_sb[hh * 64:(hh + 1) * 64, hp, :], op[0:64, :], rbs)
            if tail_cb is not None:
                for qb in range(QB):
                    tail_cb(qb)

        # prologue: all Q units fill the x^T/Wk/Wv DMA wait, then K(0) and
        # the first V ic0 blocks
        for ib in range(IB):
            q_unit(ib)
        for kc in range(KC):
            k_unit(0, kc)
        for kb in range(4):
            v_unit(kb, 0)
        for h in range(H):
            head_attn(h)
        for qb in range(QB):                     # pair 6: fills the PE while
            for dc in range(2):                  # h15's softmax tail drains
                o_unit(6, qb, dc)
        for qb in range(QB):                     # pair 7 combined, K=128
            for dc in range(2):
                o_unit(7, qb, dc)
        wo_state["pool"].release()

    nc.compile()
    return nc


def _get_nc():
    if "nc" not in _cached:
        _cached["nc"] = _build()
    return _cached["nc"]


def kernel(queries, Wq, Wkv, Wo, bo, _trace=False):
    import ml_dtypes
    from concourse.bass_utils import run_bass_kernel_spmd

    queries = np.asarray(queries, dtype=np.float32)
    bo = np.asarray(bo, dtype=np.float32)
    cast = lambda a: np.ascontiguousarray(a).astype(ml_dtypes.bfloat16)
    Wq_c = cast(Wq)
    Wk_c = cast(np.asarray(Wkv)[:, 0:INNER])
    Wv_c = cast(np.asarray(Wkv)[:, INNER:2 * INNER])
    Wo_c = cast(Wo)

    nc = _get_nc()

    in_maps = []
    for c in range(NCORES):
        g, r = c // 4, c % 4
        xT = np.ascontiguousarray(queries[g].T)          # [DM, N]
        xT = cast(np.roll(xT, -r * QR, axis=1))
        in_maps.append({"xT": xT, "Wq": Wq_c, "Wk": Wk_c, "Wv": Wv_c,
                        "Wo": Wo_c, "bo": bo})

    res = run_bass_kernel_spmd(nc, in_maps, list(range(NCORES)),
                               trace=_trace)
    out = np.empty((B, N, DM), dtype=np.float32)
    for c in range(NCORES):
        g, r = c // 4, c % 4
        out[g, r * QR:(r + 1) * QR, :] = res.results[c]["out"]
    if _trace:
        return out, res
    return out


if __name__ == "__main__":
    rng = np.random.default_rng(0)
    q = rng.standard_normal((B, N, DM), dtype=np.float32)
    s = 0.02
    inputs = dict(
        queries=q,
        Wq=(rng.standard_normal((DM, INNER), dtype=np.float32) * s),
        Wkv=(rng.standard_normal((DM, 2 * INNER), dtype=np.float32) * s),
        Wo=(rng.standard_normal((INNER, DM), dtype=np.float32) * s),
        bo=(rng.standard_normal((DM,), dtype=np.float32) * s),
    )
    out = kernel(**inputs)
    print("kernel ran, out shape", out.shape)
